# revision 9
# baseline (speedup 1.0000x reference)
"""Causal core-attention kernel for Trainium2, 8-core SPMD.

Problem: q,k,v [2048, 2, 16, 128] fp32, causal mask, softmax(QK^T/sqrt(128)) @ V,
output [2048, 2, 2048] fp32.

Sharding: the 32 (batch, head) pairs are split 4-per-core across 8 NeuronCores.
No cross-core communication.

Per-core algorithm (per (b,h) pair), flash-style but without max subtraction
(scores have unit variance so exp never overflows; the reference's -10000 mask
fill underflows to exactly 0 in fp32, so masked positions contribute 0):

  for each s-tile (512 queries), for each needed key block (128 keys):
    MM1 (TensorE, fp16):  S^T[128 t, 512 s] = K_blk^T.T @ Q^T   (contract d=128)
    exp (ScalarE):        P^T = exp(S^T * 1/sqrt(128))  -> fp16 SBUF
    mask (VectorE):       P^T *= pattern  (only blocks crossing the diagonal)
    MM2 (TensorE, fp16):  acc[128 s, 130] += P_sub^T.T @ [V_blk | 1 | 0]
                          (col 128 accumulates the softmax denominator)
  normalize (VectorE):    out = acc[:, :128] * (1 / acc[:, 128])

The mask input is inspected on the host: 128x128 blocks are classified
full/empty/partial; empty blocks are skipped entirely, partial blocks get a
deduplicated 0/1 pattern multiply. A causal mask yields 4 unique patterns and
half the compute; an all-false mask yields zero patterns and full compute.
"""

import math

import numpy as np

import concourse.bass as bass
import concourse.mybir as mybir
import concourse.tile as tile
from concourse.bass_utils import run_bass_kernel_spmd

SQ, B, NH, HN = 2048, 2, 16, 128
N_CORES = 8
N_PAIRS = (B * NH) // N_CORES  # 4 (b,h) pairs per core
TB = 128  # key-block size (t)
VW = HN + 2  # V padded with a ones column (denominator) + a zero column (even width)
SCALE = 1.0 / math.sqrt(HN)

FULL, EMPTY, PARTIAL = 0, 1, 2

_last_results = None  # BassKernelResults of the most recent kernel() call

f16 = mybir.dt.float16
f32 = mybir.dt.float32


def _classify_mask(allowed: np.ndarray, sq: int, sblk: int):
    """Host-side mask analysis. allowed[s, t] True where attention is permitted.

    Returns a schedule dict consumed by _build_program.
    """
    nsb = sq // TB  # 128-row s blocks (sigma)
    ntb = sq // TB  # 128-col t blocks (j)
    nsub = sblk // TB  # s sub-blocks per s-tile
    nst = sq // sblk  # s tiles

    st = allowed.reshape(nsb, TB, ntb, TB)
    blk_all = st.all(axis=(1, 3))  # [sigma, j]
    blk_any = st.any(axis=(1, 3))
    status = np.where(blk_all, FULL, np.where(blk_any, PARTIAL, EMPTY))

    patterns: list[np.ndarray] = []
    pat_index: dict[bytes, int] = {}
    needed_js: list[list[int]] = []
    pat_of: dict[tuple[int, int], int] = {}  # (i, j) -> pattern idx
    for i in range(nst):
        sigmas = range(i * nsub, (i + 1) * nsub)
        js = [j for j in range(ntb) if any(status[g, j] != EMPTY for g in sigmas)]
        needed_js.append(js)
        for j in js:
            if any(status[g, j] == PARTIAL for g in sigmas):
                pat = np.ascontiguousarray(
                    allowed[i * sblk : (i + 1) * sblk, j * TB : (j + 1) * TB].T
                ).astype(np.float16)
                key = pat.tobytes()
                if key not in pat_index:
                    pat_index[key] = len(patterns)
                    patterns.append(pat)
                pat_of[(i, j)] = pat_index[key]

    # first/last non-empty j per 128-row s block (for PSUM start/stop flags)
    first_j = np.full(nsb, -1, np.int64)
    last_j = np.full(nsb, -1, np.int64)
    for g in range(nsb):
        js = [j for j in range(ntb) if status[g, j] != EMPTY]
        if js:
            first_j[g], last_j[g] = js[0], js[-1]

    pats_host = None
    if patterns:
        # [TB partitions, n_pat, sblk] contiguous for a clean DMA
        pats_host = np.ascontiguousarray(np.stack(patterns, axis=0).transpose(1, 0, 2))

    return dict(
        status=status,
        needed_js=needed_js,
        pat_of=pat_of,
        first_j=first_j,
        last_j=last_j,
        pats_host=pats_host,
        nst=nst,
        nsub=nsub,
        nsb=nsb,
    )


def _split_multiwaits(nc):
    """The walrus build in this container supports exactly one sync-wait per
    instruction (NEURON_ISA_TPB_EVENTS has a single wait slot) and does not
    split multi-wait instructions itself. Tile emits instructions with several
    waits; lower each extra wait onto a same-engine NoOp carrier inserted
    immediately before the instruction (identical stall point, no reordering).
    """
    n_new = 0
    for blk in nc.m.functions[0].blocks:
        insts = blk.instructions
        i = 0
        while i < len(insts):
            ins = insts[i]
            si = ins.sync_info
            if si is not None and len(si.on_wait) > 1:
                waits = list(si.on_wait)
                carriers = []
                for w in waits[:-1]:
                    n_new += 1
                    carriers.append(
                        mybir.InstNoOp(
                            name=f"I-swsplit-{n_new}",
                            engine=ins.engine,
                            ins=[],
                            outs=[],
                            sync_info=mybir.SyncInfo(on_wait=[w], on_update=[]),
                        )
                    )
                ins.sync_info = mybir.SyncInfo(
                    on_wait=[waits[-1]], on_update=list(si.on_update)
                )
                insts[i:i] = carriers
                i += len(carriers)
            i += 1
    return n_new


def _build_program(sched, sq: int, sblk: int, n_pairs: int, repeat: int = 1):
    n_pat = 0 if sched["pats_host"] is None else sched["pats_host"].shape[1]
    status = sched["status"]
    first_j = sched["first_j"]
    last_j = sched["last_j"]
    nst, nsub, nsb = sched["nst"], sched["nsub"], sched["nsb"]
    nblk = sq // TB

    nc = bass.Bass(
        "TRN2", target_bir_lowering=False, debug=False, num_devices=N_CORES
    )
    qT = nc.dram_tensor("q_t", [n_pairs, HN, sq], f16, kind="ExternalInput").ap()
    kT = nc.dram_tensor("k_t", [n_pairs, HN, sq], f16, kind="ExternalInput").ap()
    vaug = nc.dram_tensor(
        "v_aug", [n_pairs, TB, nblk, VW], f16, kind="ExternalInput"
    ).ap()
    pats = None
    if n_pat:
        pats = nc.dram_tensor(
            "pats", [TB, n_pat, sblk], f16, kind="ExternalInput"
        ).ap()
    out = nc.dram_tensor(
        "out_ctx", [n_pairs, nsb, TB, HN], f32, kind="ExternalOutput"
    ).ap()

    with tile.TileContext(nc) as tc:
        with (
            tc.tile_pool(name="inputs", bufs=2) as inputs,
            tc.tile_pool(name="consts", bufs=1) as consts,
            tc.tile_pool(name="ptp", bufs=3) as ptp,
            tc.tile_pool(name="outp", bufs=4) as outp,
            tc.tile_pool(name="stp", bufs=2, space="PSUM") as stp,
            tc.tile_pool(name="accp", bufs=1, space="PSUM") as accp,
        ):
            pat_t = None
            if n_pat:
                pat_t = consts.tile([TB, n_pat, sblk], f16)
                nc.sync.dma_start(out=pat_t, in_=pats)

            for rep in range(repeat):
              for p in range(n_pairs):
                q_t = inputs.tile([HN, sq], f16, tag="q")
                k_t = inputs.tile([HN, sq], f16, tag="k")
                v_t = inputs.tile([TB, nblk, VW], f16, tag="v")
                nc.sync.dma_start(out=q_t, in_=qT[p])
                nc.sync.dma_start(out=k_t, in_=kT[p])
                nc.sync.dma_start(out=v_t, in_=vaug[p])

                for i in range(nst):
                    js = sched["needed_js"][i]
                    accs = [
                        accp.tile([TB, VW], f32, tag=f"acc{u}", name=f"acc_{p}_{i}_{u}")
                        for u in range(nsub)
                    ]
                    # groups of 2 key blocks share one PSUM tile / one exp
                    for g0 in range(0, len(js), 2):
                        grp = js[g0 : g0 + 2]
                        ng = len(grp)
                        st_t = stp.tile([TB, 2, sblk], f32, tag="st", name=f"st_{p}_{i}_{g0}")
                        for idx, j in enumerate(grp):
                            nc.tensor.matmul(
                                st_t[:, idx, :],
                                lhsT=k_t[:, j * TB : (j + 1) * TB],
                                rhs=q_t[:, i * sblk : (i + 1) * sblk],
                                start=True,
                                stop=True,
                            )
                        pt = ptp.tile([TB, 2, sblk], f16, tag="pt", name=f"pt_{p}_{i}_{g0}")
                        nc.scalar.activation(
                            pt[:, :ng, :],
                            st_t[:, :ng, :],
                            mybir.ActivationFunctionType.Exp,
                            scale=SCALE,
                        )
                        for idx, j in enumerate(grp):
                            pi = sched["pat_of"].get((i, j))
                            if pi is not None:
                                nc.vector.tensor_mul(
                                    pt[:, idx, :], pt[:, idx, :], pat_t[:, pi, :]
                                )
                            for u in range(nsub):
                                sig = i * nsub + u
                                if status[sig, j] == EMPTY:
                                    continue
                                nc.tensor.matmul(
                                    accs[u],
                                    lhsT=pt[:, idx, u * TB : (u + 1) * TB],
                                    rhs=v_t[:, j, :],
                                    start=(j == first_j[sig]),
                                    stop=(j == last_j[sig]),
                                )
                    for u in range(nsub):
                        sig = i * nsub + u
                        rec = outp.tile([TB, 1], f32, tag="rec", name=f"rec_{p}_{i}_{u}")
                        nc.vector.reciprocal(rec, accs[u][:, HN : HN + 1])
                        ot = outp.tile([TB, HN], f32, tag="ot", name=f"ot_{p}_{i}_{u}")
                        nc.vector.tensor_scalar_mul(ot, accs[u][:, 0:HN], rec)
                        nc.sync.dma_start(out=out[p, sig], in_=ot)
    _split_multiwaits(nc)
    return nc


def _prep_inputs(query_layer, key_layer, value_layer, sq, n_pairs_total):
    """Transpose + cast on host into DMA-friendly per-pair layouts."""
    # [s, b, nh, hn] -> [pair, hn, s] (d-major, contraction on partitions)
    qT = np.ascontiguousarray(
        query_layer.transpose(1, 2, 3, 0).reshape(n_pairs_total, HN, sq)
    ).astype(np.float16)
    kT = np.ascontiguousarray(
        key_layer.transpose(1, 2, 3, 0).reshape(n_pairs_total, HN, sq)
    ).astype(np.float16)
    # [s, b, nh, hn] -> [pair, s, hn] -> augmented [pair, TB, nblk, VW]
    v = np.ascontiguousarray(
        value_layer.transpose(1, 2, 0, 3).reshape(n_pairs_total, sq, HN)
    )
    nblk = sq // TB
    va = np.zeros((n_pairs_total, sq, VW), np.float16)
    va[:, :, :HN] = v
    va[:, :, HN] = 1.0
    vaug = np.ascontiguousarray(
        va.reshape(n_pairs_total, nblk, TB, VW).transpose(0, 2, 1, 3)
    )
    return qT, kT, vaug


def kernel(query_layer, key_layer, value_layer, attention_mask):
    sq = query_layer.shape[0]
    assert query_layer.shape == (sq, B, NH, HN)
    sblk = 512
    n_pairs_total = B * NH

    allowed = ~np.asarray(attention_mask).reshape(sq, sq)
    sched = _classify_mask(allowed, sq, sblk)
    nc = _build_program(sched, sq, sblk, N_PAIRS)

    qT, kT, vaug = _prep_inputs(
        query_layer, key_layer, value_layer, sq, n_pairs_total
    )

    in_maps = []
    for c in range(N_CORES):
        m = {
            "q_t": np.ascontiguousarray(qT[c * N_PAIRS : (c + 1) * N_PAIRS]),
            "k_t": np.ascontiguousarray(kT[c * N_PAIRS : (c + 1) * N_PAIRS]),
            "v_aug": np.ascontiguousarray(vaug[c * N_PAIRS : (c + 1) * N_PAIRS]),
        }
        if sched["pats_host"] is not None:
            m["pats"] = sched["pats_host"]
        in_maps.append(m)

    import os

    trace = os.environ.get("ATTN_TRACE", "0") == "1"
    res = run_bass_kernel_spmd(
        nc, in_maps, core_ids=list(range(N_CORES)), trace=trace
    )
    global _last_results
    _last_results = res

    out = np.empty((sq, B, NH * HN), np.float32)
    for c in range(N_CORES):
        arr = res.results[c]["out_ctx"]  # [N_PAIRS, nsb, TB, HN]
        for p in range(N_PAIRS):
            b, h = divmod(c * N_PAIRS + p, NH)
            out[:, b, h * HN : (h + 1) * HN] = arr[p].reshape(sq, HN)
    return out


# ---------------------------------------------------------------------------
# Timing utilities (dev-only; the graded path is kernel() above).
# The axon NTFF profiling hook is unavailable in this container, so we measure
# device time by wall-clocking a persistent jitted executable and differencing
# two programs that repeat the compute R1 vs R2 times (constant dispatch/RPC
# overhead cancels).
# ---------------------------------------------------------------------------


def _make_runner(nc, in_maps):
    import jax
    import concourse.mybir as _mybir
    from concourse.bass2jax import (
        _bass_exec_p,
        install_neuronx_cc_hook,
        partition_id_tensor,
    )
    from jax.experimental.shard_map import shard_map
    from jax.sharding import Mesh, NamedSharding, PartitionSpec

    install_neuronx_cc_hook()
    n_cores = len(in_maps)
    partition_name = nc.partition_id_tensor.name if nc.partition_id_tensor else None
    in_names, out_names, out_avals, zero_outs = [], [], [], []
    for alloc in nc.m.functions[0].allocations:
        if not isinstance(alloc, mybir.MemoryLocationSet):
            continue
        name = alloc.memorylocations[0].name
        if alloc.kind == "ExternalInput":
            if name != partition_name:
                in_names.append(name)
        elif alloc.kind == "ExternalOutput":
            out_names.append(name)
            shape = tuple(alloc.tensor_shape)
            dtype = _mybir.dt.np(alloc.dtype)
            out_avals.append(jax.core.ShapedArray(shape, dtype))
            zero_outs.append(np.zeros(shape, dtype))
    n_params = len(in_names)
    all_in_names = in_names + out_names
    if partition_name is not None:
        all_in_names.append(partition_name)

    def _body(*args):
        operands = list(args)
        if partition_name is not None:
            operands.append(partition_id_tensor())
        outs = _bass_exec_p.bind(
            *operands,
            out_avals=tuple(out_avals),
            in_names=tuple(all_in_names),
            out_names=tuple(out_names),
            lowering_input_output_aliases=(),
            sim_require_finite=True,
            sim_require_nnan=True,
            nc=nc,
        )
        return tuple(outs)

    devices = jax.devices()[:n_cores]
    mesh = Mesh(np.asarray(devices), ("core",))
    spec = PartitionSpec("core")
    sharded = jax.jit(
        shard_map(
            _body,
            mesh=mesh,
            in_specs=(spec,) * (n_params + len(out_names)),
            out_specs=(spec,) * len(out_names),
            check_rep=False,
        ),
        keep_unused=True,
    )
    sh = NamedSharding(mesh, spec)
    dev_in = [
        jax.device_put(
            np.concatenate([in_maps[c][n] for c in range(n_cores)], axis=0), sh
        )
        for n in in_names
    ]
    dev_zero = [
        jax.device_put(np.zeros((n_cores * z.shape[0], *z.shape[1:]), z.dtype), sh)
        for z in zero_outs
    ]

    def run():
        return jax.block_until_ready(sharded(*dev_in, *dev_zero))

    return run


def measure_exec_ns(inputs, r1=1, r2=4, iters=8):
    import time

    sq = inputs["query_layer"].shape[0]
    sblk = 512
    allowed = ~np.asarray(inputs["attention_mask"]).reshape(sq, sq)
    sched = _classify_mask(allowed, sq, sblk)
    qT, kT, vaug = _prep_inputs(
        inputs["query_layer"], inputs["key_layer"], inputs["value_layer"], sq, B * NH
    )
    in_maps = []
    for c in range(N_CORES):
        m = {
            "q_t": np.ascontiguousarray(qT[c * N_PAIRS : (c + 1) * N_PAIRS]),
            "k_t": np.ascontiguousarray(kT[c * N_PAIRS : (c + 1) * N_PAIRS]),
            "v_aug": np.ascontiguousarray(vaug[c * N_PAIRS : (c + 1) * N_PAIRS]),
        }
        if sched["pats_host"] is not None:
            m["pats"] = sched["pats_host"]
        in_maps.append(m)

    walls = {}
    for r in (r1, r2):
        nc = _build_program(sched, sq, sblk, N_PAIRS, repeat=r)
        run = _make_runner(nc, in_maps)
        run()  # compile + warm
        best = float("inf")
        for _ in range(iters):
            t0 = time.perf_counter()
            run()
            best = min(best, time.perf_counter() - t0)
        walls[r] = best
        print(f"repeat={r}: best wall {best * 1e6:.1f} us")
    per_rep_s = (walls[r2] - walls[r1]) / (r2 - r1)
    return per_rep_s * 1e9


# revision 25
# speedup vs baseline: 116.5918x; 116.5918x over previous
"""Causal core-attention kernel for Trainium2, 8-core SPMD.

Problem: q,k,v [2048, 2, 16, 128] fp32, causal mask, softmax(QK^T/sqrt(128)) @ V,
output [2048, 2, 2048] fp32.

Sharding: the 32 (batch, head) pairs are split 4-per-core across 8 NeuronCores.
No cross-core communication.

Per-core algorithm (per (b,h) pair), flash-style but without max subtraction
(scores have unit variance so exp never overflows; the reference's -10000 mask
fill underflows to exactly 0 in fp32, so masked positions contribute 0):

  for each s-tile (512 queries), for each needed key block (128 keys):
    MM1 (TensorE, fp16):  S^T[128 t, 512 s] = K_blk^T.T @ Q^T   (contract d=128)
    exp (ScalarE):        P^T = exp(S^T * 1/sqrt(128))  -> fp16 SBUF
    mask (VectorE):       P^T *= pattern  (only blocks crossing the diagonal)
    MM2 (TensorE, fp16):  acc[128 s, 130] += P_sub^T.T @ [V_blk | 1 | 0]
                          (col 128 accumulates the softmax denominator)
  normalize (VectorE):    out = acc[:, :128] * (1 / acc[:, 128])

The mask input is inspected on the host: 128x128 blocks are classified
full/empty/partial; empty blocks are skipped entirely, partial blocks get a
deduplicated 0/1 pattern multiply. A causal mask yields 4 unique patterns and
half the compute; an all-false mask yields zero patterns and full compute.
"""

import math

import ml_dtypes
import numpy as np

import concourse.bass as bass
import concourse.mybir as mybir
import concourse.tile as tile
from concourse.bass_utils import run_bass_kernel_spmd

SQ, B, NH, HN = 2048, 2, 16, 128
N_CORES = 8
N_PAIRS = (B * NH) // N_CORES  # 4 (b,h) pairs per core
TB = 128  # key-block size (t)
VW = HN + 2  # V padded with a ones column (denominator) + a zero column (even width)
SCALE = 1.0 / math.sqrt(HN)

FULL, EMPTY, PARTIAL = 0, 1, 2

_last_results = None  # BassKernelResults of the most recent kernel() call

f16 = mybir.dt.bfloat16
f32 = mybir.dt.float32
_np16 = ml_dtypes.bfloat16


def _classify_mask(allowed: np.ndarray, sq: int, sblk: int):
    """Host-side mask analysis. allowed[s, t] True where attention is permitted.

    Returns a schedule dict consumed by _build_program.
    """
    nsb = sq // TB  # 128-row s blocks (sigma)
    ntb = sq // TB  # 128-col t blocks (j)
    nsub = sblk // TB  # s sub-blocks per s-tile
    nst = sq // sblk  # s tiles

    st = allowed.reshape(nsb, TB, ntb, TB)
    blk_all = st.all(axis=(1, 3))  # [sigma, j]
    blk_any = st.any(axis=(1, 3))
    status = np.where(blk_all, FULL, np.where(blk_any, PARTIAL, EMPTY))

    patterns: list[np.ndarray] = []
    pat_index: dict[bytes, int] = {}
    needed_js: list[list[int]] = []
    pat_of: dict[tuple[int, int], int] = {}  # (i, j) -> pattern idx
    for i in range(nst):
        sigmas = range(i * nsub, (i + 1) * nsub)
        js = [j for j in range(ntb) if any(status[g, j] != EMPTY for g in sigmas)]
        needed_js.append(js)
        for j in js:
            if any(status[g, j] == PARTIAL for g in sigmas):
                pat = np.ascontiguousarray(
                    allowed[i * sblk : (i + 1) * sblk, j * TB : (j + 1) * TB].T
                ).astype(_np16)
                key = pat.tobytes()
                if key not in pat_index:
                    pat_index[key] = len(patterns)
                    patterns.append(pat)
                pat_of[(i, j)] = pat_index[key]

    # first/last non-empty j per 128-row s block (for PSUM start/stop flags)
    first_j = np.full(nsb, -1, np.int64)
    last_j = np.full(nsb, -1, np.int64)
    for g in range(nsb):
        js = [j for j in range(ntb) if status[g, j] != EMPTY]
        if js:
            first_j[g], last_j[g] = js[0], js[-1]

    pats_host = None
    if patterns:
        # [TB partitions, n_pat, sblk] contiguous for a clean DMA
        pats_host = np.ascontiguousarray(np.stack(patterns, axis=0).transpose(1, 0, 2))

    return dict(
        status=status,
        needed_js=needed_js,
        pat_of=pat_of,
        first_j=first_j,
        last_j=last_j,
        pats_host=pats_host,
        nst=nst,
        nsub=nsub,
        nsb=nsb,
    )


def _split_multiwaits(nc):
    """The walrus build in this container supports exactly one sync-wait per
    instruction (NEURON_ISA_TPB_EVENTS has a single wait slot) and does not
    split multi-wait instructions itself. Tile emits instructions with several
    waits; lower each extra wait onto a same-engine NoOp carrier inserted
    immediately before the instruction (identical stall point, no reordering).
    """
    n_new = 0
    for blk in nc.m.functions[0].blocks:
        insts = blk.instructions
        i = 0
        while i < len(insts):
            ins = insts[i]
            si = ins.sync_info
            if si is not None and len(si.on_wait) > 1:
                waits = list(si.on_wait)
                carriers = []
                for w in waits[:-1]:
                    n_new += 1
                    carriers.append(
                        mybir.InstNoOp(
                            name=f"I-swsplit-{n_new}",
                            engine=ins.engine,
                            ins=[],
                            outs=[],
                            sync_info=mybir.SyncInfo(on_wait=[w], on_update=[]),
                        )
                    )
                ins.sync_info = mybir.SyncInfo(
                    on_wait=[waits[-1]], on_update=list(si.on_update)
                )
                insts[i:i] = carriers
                i += len(carriers)
            i += 1
    return n_new


def _build_program(sched, sq: int, sblk: int, n_pairs: int, repeat: int = 1):
    n_pat = 0 if sched["pats_host"] is None else sched["pats_host"].shape[1]
    status = sched["status"]
    first_j = sched["first_j"]
    last_j = sched["last_j"]
    nst, nsub, nsb = sched["nst"], sched["nsub"], sched["nsb"]
    nblk = sq // TB

    nc = bass.Bass(
        "TRN2", target_bir_lowering=False, debug=False, num_devices=N_CORES
    )
    W = 2 * sq + nblk * VW  # concatenated [Q^T | K^T | V_aug] width per pair
    qkv = nc.dram_tensor("qkv", [n_pairs, TB, W], f16, kind="ExternalInput").ap()
    pats = None
    if n_pat:
        pats = nc.dram_tensor(
            "pats", [TB, n_pat, sblk], f16, kind="ExternalInput"
        ).ap()
    out = nc.dram_tensor(
        "out_ctx", [n_pairs, nst, TB, nsub * HN], f32, kind="ExternalOutput"
    ).ap()

    with tile.TileContext(nc) as tc:
        GRP = 2  # key blocks per PSUM tile / exp instruction
        with (
            tc.tile_pool(name="inputs", bufs=2) as inputs,
            tc.tile_pool(name="consts", bufs=1) as consts,
            tc.tile_pool(name="ptp", bufs=4) as ptp,
            tc.tile_pool(name="outp", bufs=4) as outp,
            tc.tile_pool(name="stp", bufs=2, space="PSUM") as stp,
            tc.tile_pool(name="accp", bufs=1, space="PSUM") as accp,
        ):
            pat_t = None
            if n_pat:
                pat_t = consts.tile([TB, n_pat, sblk], f16)
                nc.sync.dma_start(out=pat_t, in_=pats)

            for rep in range(repeat):
              for p in range(n_pairs):
                qkv_t = inputs.tile([TB, W], f16, tag="qkv")
                nc.sync.dma_start(out=qkv_t, in_=qkv[p])
                q_t = qkv_t[:, 0:sq]
                k_t = qkv_t[:, sq : 2 * sq]
                v_t = qkv_t[:, 2 * sq :].rearrange("p (j c) -> p j c", c=VW)

                for i in range(nst):
                    js = sched["needed_js"][i]
                    # one PSUM bank per accumulator: a start=True matmul clears
                    # has_written for its whole bank, so groups must not share
                    accs = [
                        accp.tile([TB, VW], f32, tag=f"acc{u}", name=f"acc_{p}_{i}_{u}")
                        for u in range(nsub)
                    ]
                    # groups of GRP key blocks share one PSUM tile / one exp
                    for g0 in range(0, len(js), GRP):
                        grp = js[g0 : g0 + GRP]
                        ng = len(grp)
                        st_t = stp.tile([TB, GRP, sblk], f32, tag="st", name=f"st_{p}_{i}_{g0}")
                        for idx, j in enumerate(grp):
                            nc.tensor.matmul(
                                st_t[:, idx, :],
                                lhsT=k_t[:, j * TB : (j + 1) * TB],
                                rhs=q_t[:, i * sblk : (i + 1) * sblk],
                                start=True,
                                stop=True,
                            )
                        pt = ptp.tile([TB, GRP, sblk], f16, tag="pt", name=f"pt_{p}_{i}_{g0}")
                        nc.scalar.activation(
                            pt[:, :ng, :],
                            st_t[:, :ng, :],
                            mybir.ActivationFunctionType.Exp,
                            scale=SCALE,
                        )
                        for idx, j in enumerate(grp):
                            pi = sched["pat_of"].get((i, j))
                            if pi is not None:
                                nc.vector.tensor_mul(
                                    pt[:, idx, :], pt[:, idx, :], pat_t[:, pi, :]
                                )
                            for u in range(nsub):
                                sig = i * nsub + u
                                if status[sig, j] == EMPTY:
                                    continue
                                nc.tensor.matmul(
                                    accs[u],
                                    lhsT=pt[:, idx, u * TB : (u + 1) * TB],
                                    rhs=v_t[:, j, :],
                                    start=(j == first_j[sig]),
                                    stop=(j == last_j[sig]),
                                )
                    ot = outp.tile([TB, nsub, HN], f32, tag="ot", name=f"ot_{p}_{i}")
                    for u in range(nsub):
                        rec = outp.tile([TB, 1], f32, tag="rec", name=f"rec_{p}_{i}_{u}")
                        nc.vector.reciprocal(rec, accs[u][:, HN : HN + 1])
                        nc.vector.tensor_scalar_mul(ot[:, u, :], accs[u][:, 0:HN], rec)
                    nc.gpsimd.dma_start(out=out[p, i], in_=ot)
    _split_multiwaits(nc)
    return nc


def _prep_inputs(query_layer, key_layer, value_layer, sq, n_pairs_total):
    """Transpose + cast on host into one DMA-friendly concatenated layout:
    per pair, [128 partitions, 2*sq + nblk*VW] = [Q^T | K^T | V_aug]."""
    nblk = sq // TB
    W = 2 * sq + nblk * VW
    qkv = np.empty((n_pairs_total, TB, W), _np16)
    # [s, b, nh, hn] -> [pair, hn, s] (d-major, contraction on partitions)
    qkv[:, :, 0:sq] = query_layer.transpose(1, 2, 3, 0).reshape(
        n_pairs_total, HN, sq
    )
    qkv[:, :, sq : 2 * sq] = key_layer.transpose(1, 2, 3, 0).reshape(
        n_pairs_total, HN, sq
    )
    # [s, b, nh, hn] -> [pair, s, hn] -> augmented, t-in-block on partitions
    v = value_layer.transpose(1, 2, 0, 3).reshape(n_pairs_total, sq, HN)
    va = np.zeros((n_pairs_total, sq, VW), _np16)
    va[:, :, :HN] = v
    va[:, :, HN] = 1.0
    qkv[:, :, 2 * sq :] = (
        va.reshape(n_pairs_total, nblk, TB, VW)
        .transpose(0, 2, 1, 3)
        .reshape(n_pairs_total, TB, nblk * VW)
    )
    return qkv


def kernel(query_layer, key_layer, value_layer, attention_mask):
    sq = query_layer.shape[0]
    assert query_layer.shape == (sq, B, NH, HN)
    sblk = 512
    n_pairs_total = B * NH

    allowed = ~np.asarray(attention_mask).reshape(sq, sq)
    sched = _classify_mask(allowed, sq, sblk)
    nc = _build_program(sched, sq, sblk, N_PAIRS)

    qkv = _prep_inputs(query_layer, key_layer, value_layer, sq, n_pairs_total)

    in_maps = []
    for c in range(N_CORES):
        m = {"qkv": np.ascontiguousarray(qkv[c * N_PAIRS : (c + 1) * N_PAIRS])}
        if sched["pats_host"] is not None:
            m["pats"] = sched["pats_host"]
        in_maps.append(m)

    import os

    trace = os.environ.get("ATTN_TRACE", "0") == "1"
    res = run_bass_kernel_spmd(
        nc, in_maps, core_ids=list(range(N_CORES)), trace=trace
    )
    global _last_results
    _last_results = res

    out = np.empty((sq, B, NH * HN), np.float32)
    nst, nsub = sched["nst"], sched["nsub"]
    for c in range(N_CORES):
        arr = res.results[c]["out_ctx"]  # [N_PAIRS, nst, TB, nsub*HN]
        for p in range(N_PAIRS):
            b, h = divmod(c * N_PAIRS + p, NH)
            # [nst, TB(r), nsub(u), HN] -> s = i*sblk + u*TB + r
            o = arr[p].reshape(nst, TB, nsub, HN).transpose(0, 2, 1, 3)
            out[:, b, h * HN : (h + 1) * HN] = o.reshape(sq, HN)
    return out


# ---------------------------------------------------------------------------
# Timing utilities (dev-only; the graded path is kernel() above).
# The axon NTFF profiling hook is unavailable in this container, so we measure
# device time by wall-clocking a persistent jitted executable and differencing
# two programs that repeat the compute R1 vs R2 times (constant dispatch/RPC
# overhead cancels).
# ---------------------------------------------------------------------------


def _make_runner(nc, in_maps):
    import jax
    import concourse.mybir as _mybir
    from concourse.bass2jax import (
        _bass_exec_p,
        install_neuronx_cc_hook,
        partition_id_tensor,
    )
    from jax.experimental.shard_map import shard_map
    from jax.sharding import Mesh, NamedSharding, PartitionSpec

    install_neuronx_cc_hook()
    n_cores = len(in_maps)
    partition_name = nc.partition_id_tensor.name if nc.partition_id_tensor else None
    in_names, out_names, out_avals, zero_outs = [], [], [], []
    for alloc in nc.m.functions[0].allocations:
        if not isinstance(alloc, mybir.MemoryLocationSet):
            continue
        name = alloc.memorylocations[0].name
        if alloc.kind == "ExternalInput":
            if name != partition_name:
                in_names.append(name)
        elif alloc.kind == "ExternalOutput":
            out_names.append(name)
            shape = tuple(alloc.tensor_shape)
            dtype = _mybir.dt.np(alloc.dtype)
            out_avals.append(jax.core.ShapedArray(shape, dtype))
            zero_outs.append(np.zeros(shape, dtype))
    n_params = len(in_names)
    all_in_names = in_names + out_names
    if partition_name is not None:
        all_in_names.append(partition_name)

    def _body(*args):
        operands = list(args)
        if partition_name is not None:
            operands.append(partition_id_tensor())
        outs = _bass_exec_p.bind(
            *operands,
            out_avals=tuple(out_avals),
            in_names=tuple(all_in_names),
            out_names=tuple(out_names),
            lowering_input_output_aliases=(),
            sim_require_finite=True,
            sim_require_nnan=True,
            nc=nc,
        )
        return tuple(outs)

    devices = jax.devices()[:n_cores]
    mesh = Mesh(np.asarray(devices), ("core",))
    spec = PartitionSpec("core")
    sharded = jax.jit(
        shard_map(
            _body,
            mesh=mesh,
            in_specs=(spec,) * (n_params + len(out_names)),
            out_specs=(spec,) * len(out_names),
            check_rep=False,
        ),
        keep_unused=True,
    )
    sh = NamedSharding(mesh, spec)
    dev_in = [
        jax.device_put(
            np.concatenate([in_maps[c][n] for c in range(n_cores)], axis=0), sh
        )
        for n in in_names
    ]
    dev_zero = [
        jax.device_put(np.zeros((n_cores * z.shape[0], *z.shape[1:]), z.dtype), sh)
        for z in zero_outs
    ]

    def run():
        return jax.block_until_ready(sharded(*dev_in, *dev_zero))

    return run


def measure_exec_ns(inputs, r1=2, r2=12, iters=12):
    import time

    sq = inputs["query_layer"].shape[0]
    sblk = 512
    allowed = ~np.asarray(inputs["attention_mask"]).reshape(sq, sq)
    sched = _classify_mask(allowed, sq, sblk)
    qkv = _prep_inputs(
        inputs["query_layer"], inputs["key_layer"], inputs["value_layer"], sq, B * NH
    )
    in_maps = []
    for c in range(N_CORES):
        m = {"qkv": np.ascontiguousarray(qkv[c * N_PAIRS : (c + 1) * N_PAIRS])}
        if sched["pats_host"] is not None:
            m["pats"] = sched["pats_host"]
        in_maps.append(m)

    walls = {}
    for r in (r1, r2):
        nc = _build_program(sched, sq, sblk, N_PAIRS, repeat=r)
        run = _make_runner(nc, in_maps)
        run()  # compile + warm
        best = float("inf")
        for _ in range(iters):
            t0 = time.perf_counter()
            run()
            best = min(best, time.perf_counter() - t0)
        walls[r] = best
        print(f"repeat={r}: best wall {best * 1e6:.1f} us")
    per_rep_s = (walls[r2] - walls[r1]) / (r2 - r1)
    return per_rep_s * 1e9
